# revision 19
# baseline (speedup 1.0000x reference)
"""BiDiTreeLSTM forest kernel for 8 Trainium2 NeuronCores.

Strategy (data-parallel over trees, per the sharding hint):
  - 256 complete binary trees (depth 8, 511 nodes); 32 trees per core.
  - Everything on-device is FEATURE-MAJOR: [128 features on partitions,
    nodes on the free axis].  H == X == 128 exactly fills the partitions.
  - Host pre-permutes each core's nodes into a level-grouped "chunk-local
    split" order: level blocks stored t=8..0; within a level, the children
    of the parents in 512-column chunk j form two adjacent 512-column
    chunks 2j (left) and 2j+1 (right).  Every child/parent gather in both
    propagation passes is then a contiguous column slice and dependencies
    between levels are chunk-local.
  - Bottom-up then top-down level-synchronous ChildSum TreeLSTM per core,
    matmuls in bf16 (fp32 PSUM accumulate), elementwise in bf16 on the
    vector engine (2x/4x mode), gates on the scalar engine.
  - The scalar (ACT) engine is the bottleneck (~300ns fixed cost per
    instruction + ~0.85ns/col), so ACT instructions are merged:
      * tanh(u) = 2*sigmoid(2u) - 1 with the u-row blocks of W/U pre-scaled
        by 2 on the host, so i,o,u share ONE sigmoid over the contiguous
        PSUM iou tile; a 1-instruction vector affine (2x-1) recovers tanh.
      * tanh(c)/h of two adjacent chunks are deferred and flushed as ONE
        double-width tanh ("phase2 pairing").
      * top-down forget gates of two adjacent parent chunks share one
        sigmoid and one f*c multiply.
  - The 32 trees are split into TWO 16-tree streams emitted phase-shifted:
    the small tree levels (<=512 cols) are serial latency chains that
    leave every engine mostly idle, so one stream's big levels are
    interleaved into the other stream's serial middle.  Streams are
    independent; they share the PSUM pools (WAR deps serialize banks).
  - h0/c0 are zeros and b_iou_* are zeros by problem spec; the kernel
    exploits that (they do not affect the output otherwise).
Output per core: [128, 64] fp32, feature-major:
  [root h_bu (A|B 16+16) | leaf-mean h_td (A|B 16+16)];
host reassembles the [256, 256] result.
"""

import os
import sys

sys.path.insert(0, "/opt/trn_rl_repo")

import ml_dtypes
import numpy as np

import concourse.bass as bass
import concourse.mybir as mybir
import concourse.tile as tile

B = 256
DEPTH = 8
M = 511
H = 128
NCORES = 8
TPC = B // NCORES            # trees per core = 32
NS = 2                       # phase-shifted streams per core
TPCS = TPC // NS             # trees per stream = 16
NODES_S = TPCS * M           # 8176
NC_NODES = TPC * M           # 16352
CH = 512                     # moving-dim chunk (one fp32 PSUM bank)

NTS = {t: TPCS * (1 << t) for t in range(DEPTH + 1)}    # cols per level
OFFS = {}
_o = 0
for _t in range(DEPTH, -1, -1):                           # level 8 first
    OFFS[_t] = _o
    _o += NTS[_t]
assert _o == NODES_S

F32 = mybir.dt.float32
BF16 = mybir.dt.bfloat16
ActF = mybir.ActivationFunctionType
Alu = mybir.AluOpType

_NC_CACHE = {}


def _split_multi_waits(nc):
    """This container's walrus accepts at most ONE sync wait per
    instruction; Tile attaches several.  Insert single-wait NoOps."""
    n = 0
    for fn in nc.m.functions:
        for bb in fn.blocks:
            insts = bb.instructions
            new = []
            for inst in insts:
                si = inst.sync_info
                if si is not None and si.on_wait and len(si.on_wait) > 1:
                    waits = list(si.on_wait)
                    for j, w in enumerate(waits[:-1]):
                        new.append(mybir.InstNoOp(
                            name=f"{inst.name}_w{j}",
                            sync_info=mybir.SyncInfo(on_wait=[w], on_update=[]),
                            bass_nofuse=True,
                            engine=inst.engine,
                        ))
                        n += 1
                    si.on_wait = [waits[-1]]
                new.append(inst)
            if len(new) != len(insts):
                bb.instructions[:] = new
    return n


def _build_nc():
    nc = bass.Bass("TRN2")

    # weights prepended to X: [WBU | UBU | UFBU | WTDX | WTDH | UTD | UFTD |
    # row0-extras (bf_bu | bf_td | ones, for K=1 bias matmuls) | X nodes]
    # (u-gate row blocks pre-scaled by 2: tanh(u) = 2*sig(2u)-1)
    XW = 2176 + 768
    XTW = nc.dram_tensor("XTW", [H, XW + NC_NODES], BF16,
                         kind="ExternalInput")
    BPACK = nc.dram_tensor("BPACK", [H, 2], F32, kind="ExternalInput")
    OUT = nc.dram_tensor("OUT", [H, 64], F32, kind="ExternalOutput")

    with tile.TileContext(nc) as tc:
        with tc.tile_pool(name="persist", bufs=1) as P, \
             tc.tile_pool(name="ws", bufs=3) as W, \
             tc.tile_pool(name="sio", bufs=6) as SIO, \
             tc.tile_pool(name="fcl", bufs=4) as FCL, \
             tc.tile_pool(name="psiou", bufs=2, space="PSUM") as PI, \
             tc.tile_pool(name="psf", bufs=1, space="PSUM") as PF:

            # ---- persistent SBUF ----
            xtw = P.tile([H, XW + NC_NODES], BF16)
            wpk = xtw[:, 0:2176]
            xt = xtw[:, XW:]
            bpk = P.tile([H, 2], F32)
            # sync (HWDGE) queue: the level-8 weights + first chunk land
            # first so compute starts immediately; the rest of x streams on
            # the SWDGE (gpsimd) queue
            nc.sync.dma_start(out=xtw[:, 0:384], in_=XTW[:, 0:384])
            nc.sync.dma_start(out=xtw[:, XW:XW + 512],
                              in_=XTW[:, XW:XW + 512])
            nc.sync.dma_start(out=xtw[:, 384:XW], in_=XTW[:, 384:XW])
            nc.gpsimd.dma_start(out=bpk[:], in_=BPACK[:])
            for a, b in ((512, 2048), (2048, 4608), (4608, 8176),
                         (8176, 10240), (10240, 13312), (13312, NC_NODES)):
                nc.gpsimd.dma_start(out=xtw[:, XW + a:XW + b],
                                    in_=XTW[:, XW + a:XW + b])
            # K=1 matmul operands (partition row 0 only)
            bbu1 = xtw[0:1, 2176:2304]
            btd1 = xtw[0:1, 2304:2432]
            ones1 = xtw[0:1, 2432:2944]
            wbu = wpk[:, 0:384]
            ubu = wpk[:, 384:768]
            ufbu = wpk[:, 768:896]
            wtdx = wpk[:, 896:1280]
            wtdh = wpk[:, 1280:1664]
            utd = wpk[:, 1664:2048]
            uftd = wpk[:, 2048:2176]
            bfbu = bpk[:, 0:1]
            bftd = bpk[:, 1:2]

            hbu = P.tile([H, NC_NODES], BF16)     # all bottom-up h
            ca = P.tile([H, 8192], BF16)          # c ping (even levels)
            cb = P.tile([H, 4096], BF16)          # c pong (odd levels)
            ha = P.tile([H, 2048], BF16)          # td h ping (even, t<8)
            hb = P.tile([H, 4096], BF16)          # td h pong (odd)
            outsb = P.tile([H, 64], F32)

            def mk_stream(s):
                """Per-stream views and emission generators."""
                xs = xt[:, s * NODES_S:(s + 1) * NODES_S]
                hs = hbu[:, s * NODES_S:(s + 1) * NODES_S]
                ca_s = ca[:, s * 4096:(s + 1) * 4096]
                cb_s = cb[:, s * 2048:(s + 1) * 2048]
                ha_s = ha[:, s * 1024:(s + 1) * 1024]
                hb_s = hb[:, s * 2048:(s + 1) * 2048]

                def cbuf(t):
                    return ca_s if t % 2 == 0 else cb_s

                def hbuf(t):
                    return ha_s if t % 2 == 0 else hb_s

                def bu_flush(pend):
                    if len(pend) == 2 and pend[0][0] == pend[1][0] \
                            and pend[1][1] == pend[0][1] + pend[0][2]:
                        (t, c0, w, G, sa), (_, _, _, _, sb) = pend
                        tch = W.tile([H, 2 * CH], BF16, tag="tc")
                        nc.scalar.activation(out=tch[:, :2 * w],
                                             in_=cbuf(t)[:, c0: c0 + 2 * w],
                                             func=ActF.Tanh)
                        o = OFFS[t] + c0
                        nc.vector.tensor_mul(hs[:, o: o + w],
                                             sa[:, G:G + w], tch[:, :w])
                        nc.vector.tensor_mul(hs[:, o + w: o + 2 * w],
                                             sb[:, G:G + w], tch[:, w:2 * w])
                    else:
                        for t, c0, w, G, sa in pend:
                            tch = W.tile([H, 2 * CH], BF16, tag="tc")
                            nc.scalar.activation(out=tch[:, :w],
                                                 in_=cbuf(t)[:, c0: c0 + w],
                                                 func=ActF.Tanh)
                            nc.vector.tensor_mul(
                                hs[:, OFFS[t] + c0: OFFS[t] + c0 + w],
                                sa[:, G:G + w], tch[:, :w])
                            if t == 0:
                                nc.vector.tensor_mul(
                                    outsb[:, s * TPCS:(s + 1) * TPCS],
                                    sa[:, G:G + w], tch[:, :w])
                    pend.clear()

                def bu_gen():
                    pending = []
                    for t in range(DEPTH, -1, -1):
                        n = NTS[t]
                        xoff = OFFS[t]
                        cw = cbuf(t)
                        for c0 in range(0, n, CH):
                            if len(pending) == 2:
                                bu_flush(pending)
                            w = min(CH, n - c0)
                            G = w if w < CH else CH
                            iou = PI.tile([H, 3 * CH], F32, tag="iou")
                            for k in range(3):
                                nc.tensor.matmul(
                                    iou[:, k * G: k * G + w],
                                    wbu[:, k * H:(k + 1) * H],
                                    xs[:, xoff + c0: xoff + c0 + w],
                                    start=(k * G) % 512 == 0,
                                    stop=(t == DEPTH))
                            small = t < DEPTH and w <= 256
                            if t < DEPTH:
                                choff = OFFS[t + 1]
                                hl = hs[:, choff + 2 * c0:
                                        choff + 2 * c0 + w]
                                hr = hs[:, choff + 2 * c0 + w:
                                        choff + 2 * c0 + 2 * w]
                                cc = cbuf(t + 1)
                                if small:
                                    # forget-gate pre-activations ride in
                                    # the iou tile at [3w:5w] (bias added
                                    # via K=1 ones matmuls) so ONE sigmoid
                                    # covers i,o,u,fl,fr
                                    nc.tensor.matmul(
                                        iou[:, 3 * w:4 * w], ufbu, hl,
                                        start=(3 * w) % 512 == 0,
                                        stop=False, skip_group_check=True)
                                    nc.tensor.matmul(
                                        iou[:, 4 * w:5 * w], ufbu, hr,
                                        start=(4 * w) % 512 == 0,
                                        stop=False, skip_group_check=True)
                                    nc.tensor.matmul(
                                        iou[:, 3 * w:4 * w], bbu1,
                                        ones1[:, 0:w], start=False,
                                        stop=False, skip_group_check=True)
                                    nc.tensor.matmul(
                                        iou[:, 4 * w:5 * w], bbu1,
                                        ones1[:, 0:w], start=False,
                                        stop=True, skip_group_check=True)
                                else:
                                    psf = PF.tile([H, 2 * CH], F32,
                                                  tag="pf")
                                    nc.tensor.matmul(psf[:, 0:w], ufbu, hl,
                                                     start=True, stop=True)
                                    nc.tensor.matmul(psf[:, G:G + w],
                                                     ufbu, hr,
                                                     start=G % 512 == 0,
                                                     stop=True)
                                if w == n:
                                    # single-chunk level: U@(hl+hr) as two
                                    # matmuls (skips the htild hop)
                                    for k in range(3):
                                        nc.tensor.matmul(
                                            iou[:, k * G: k * G + w],
                                            ubu[:, k * H:(k + 1) * H],
                                            hl, start=False, stop=False,
                                            skip_group_check=small)
                                        nc.tensor.matmul(
                                            iou[:, k * G: k * G + w],
                                            ubu[:, k * H:(k + 1) * H],
                                            hr, start=False,
                                            stop=not small,
                                            skip_group_check=small)
                                else:
                                    htild = W.tile([H, CH], BF16,
                                                   tag="htild")
                                    nc.vector.tensor_add(htild[:, :w],
                                                         hl, hr)
                                    for k in range(3):
                                        nc.tensor.matmul(
                                            iou[:, k * G: k * G + w],
                                            ubu[:, k * H:(k + 1) * H],
                                            htild[:, :w],
                                            start=False, stop=True)
                            if t < DEPTH and not small:
                                # f sigmoid first: its psf inputs are ready
                                # before the U matmuls feeding the big
                                # sigmoid (in-order ACT queue)
                                f = W.tile([H, 2 * CH], BF16, tag="f")
                                nc.scalar.activation(out=f[:, :G + w],
                                                     in_=psf[:, :G + w],
                                                     func=ActF.Sigmoid,
                                                     bias=bfbu)
                            # ONE sigmoid over [i | o | 2u (| fl | fr)]
                            send = 5 * w if small else 2 * G + w
                            sio = SIO.tile([H, 3 * CH], BF16, tag="sio")
                            nc.scalar.activation(out=sio[:, :send],
                                                 in_=iou[:, 0:send],
                                                 func=ActF.Sigmoid)
                            if t < DEPTH:
                                fsl = sio[:, 3 * w:5 * w] if small \
                                    else f[:, :2 * w]
                                fc2 = W.tile([H, 2 * CH], BF16, tag="fc2")
                                nc.vector.tensor_mul(
                                    fc2[:, :2 * w], fsl,
                                    cc[:, 2 * c0: 2 * c0 + 2 * w])
                                cred = W.tile([H, CH], BF16, tag="cred")
                                nc.vector.tensor_add(cred[:, :w],
                                                     fc2[:, :w],
                                                     fc2[:, w:2 * w])
                            # tanh(u) = 2*sig(2u) - 1
                            tud = W.tile([H, CH], BF16, tag="tud")
                            nc.vector.tensor_scalar(
                                out=tud[:, :w],
                                in0=sio[:, 2 * G:2 * G + w],
                                scalar1=2.0, scalar2=-1.0,
                                op0=Alu.mult, op1=Alu.add)
                            cdst = cw[:, c0: c0 + w]
                            if t < DEPTH:
                                t1 = W.tile([H, CH], BF16, tag="t1")
                                nc.vector.tensor_mul(t1[:, :w], sio[:, :w],
                                                     tud[:, :w])
                                nc.vector.tensor_add(cdst, t1[:, :w],
                                                     cred[:, :w])
                            else:
                                nc.vector.tensor_mul(cdst, sio[:, :w],
                                                     tud[:, :w])
                            pending.append((t, c0, w, G, sio))
                            yield
                        if n <= 2 * CH:
                            bu_flush(pending)
                    if pending:
                        bu_flush(pending)

                def td_flush(pend):
                    paired = (len(pend) == 2 and pend[0][0] == pend[1][0]
                              and pend[1][1] == pend[0][1] + pend[0][2])
                    if paired:
                        (t, coff, w, G, sa), (_, _, _, _, sb) = pend
                        tch = W.tile([H, 2 * CH], BF16, tag="tc")
                        nc.scalar.activation(out=tch[:, :2 * w],
                                             in_=cbuf(t)[:, coff:
                                                         coff + 2 * w],
                                             func=ActF.Tanh)
                        if t < DEPTH:
                            hw_ = hbuf(t)
                            nc.vector.tensor_mul(hw_[:, coff: coff + w],
                                                 sa[:, G:G + w], tch[:, :w])
                            nc.vector.tensor_mul(
                                hw_[:, coff + w: coff + 2 * w],
                                sb[:, G:G + w], tch[:, w:2 * w])
                        else:
                            hn = W.tile([H, 2 * CH], BF16, tag="hn")
                            nc.vector.tensor_mul(hn[:, :w],
                                                 sa[:, G:G + w], tch[:, :w])
                            nc.vector.tensor_mul(hn[:, w:2 * w],
                                                 sb[:, G:G + w],
                                                 tch[:, w:2 * w])
                            # leaf-mean partial folded straight into the
                            # output accumulator
                            pi = coff // (2 * CH)
                            ob = outsb[:, 32 + s * TPCS:32 + (s + 1) * TPCS]
                            lred = W.tile([H, TPCS], F32, tag="lred")
                            nc.vector.reduce_sum(
                                out=lred[:],
                                in_=hn[:, :2 * w].rearrange(
                                    "p (k t) -> p t k", t=TPCS),
                                axis=mybir.AxisListType.X)
                            if pi == 0:
                                nc.vector.tensor_scalar_mul(
                                    ob, lred[:], 1.0 / 256.0)
                            else:
                                nc.vector.scalar_tensor_tensor(
                                    out=ob, in0=lred[:],
                                    scalar=1.0 / 256.0, in1=ob,
                                    op0=Alu.mult, op1=Alu.add)
                    else:
                        for t, coff, w, G, sa in pend:
                            tch = W.tile([H, 2 * CH], BF16, tag="tc")
                            nc.scalar.activation(out=tch[:, :w],
                                                 in_=cbuf(t)[:, coff:
                                                             coff + w],
                                                 func=ActF.Tanh)
                            nc.vector.tensor_mul(
                                hbuf(t)[:, coff: coff + w],
                                sa[:, G:G + w], tch[:, :w])
                    pend.clear()

                def td_gen():
                    pending = []
                    for t in range(0, DEPTH + 1):
                        n = NTS[t]
                        xoff = OFFS[t]
                        cw = cbuf(t)
                        if t == 0:
                            w = n  # 16
                            G = w
                            iou = PI.tile([H, 3 * CH], F32, tag="iou")
                            for k in range(3):
                                nc.tensor.matmul(
                                    iou[:, k * G: k * G + w],
                                    wtdx[:, k * H:(k + 1) * H],
                                    xs[:, xoff: xoff + w],
                                    start=(k == 0), stop=False)
                                nc.tensor.matmul(
                                    iou[:, k * G: k * G + w],
                                    wtdh[:, k * H:(k + 1) * H],
                                    hs[:, xoff: xoff + w],
                                    start=False, stop=True)
                            sio = SIO.tile([H, 3 * CH], BF16, tag="sio")
                            nc.scalar.activation(out=sio[:, :3 * w],
                                                 in_=iou[:, 0:3 * w],
                                                 func=ActF.Sigmoid)
                            tud = W.tile([H, CH], BF16, tag="tud")
                            nc.vector.tensor_scalar(
                                out=tud[:, :w],
                                in0=sio[:, 2 * G:2 * G + w],
                                scalar1=2.0, scalar2=-1.0,
                                op0=Alu.mult, op1=Alu.add)
                            nc.vector.tensor_mul(cw[:, 0:w], sio[:, :w],
                                                 tud[:, :w])
                            pending.append((0, 0, w, G, sio))
                            td_flush(pending)
                            yield
                            continue
                        half = n // 2
                        hp = hbuf(t - 1)
                        cp = cbuf(t - 1)
                        fc2td = None
                        for p0 in range(0, half, CH):
                            if len(pending) == 2:
                                td_flush(pending)
                            w = min(CH, half - p0)
                            G = w if w < CH else CH
                            small = w <= 256
                            if not small and p0 % (2 * CH) == 0:
                                # forget gates for parent chunks p0 and
                                # p0+CH share one sigmoid and one f*c mul
                                fw = min(2 * CH, half - p0)
                                psf = PF.tile([H, 2 * CH], F32, tag="pf")
                                nc.tensor.matmul(psf[:, 0:w], uftd,
                                                 hp[:, p0: p0 + w],
                                                 start=True, stop=True)
                                if fw > CH:
                                    nc.tensor.matmul(
                                        psf[:, CH:fw], uftd,
                                        hp[:, p0 + CH: p0 + fw],
                                        start=True, stop=True)
                                f = W.tile([H, 2 * CH], BF16, tag="f")
                                nc.scalar.activation(out=f[:, :fw],
                                                     in_=psf[:, :fw],
                                                     func=ActF.Sigmoid,
                                                     bias=bftd)
                                fc2td = FCL.tile([H, 2 * CH], BF16,
                                                 tag="fcl")
                                nc.vector.tensor_mul(fc2td[:, :fw],
                                                     f[:, :fw],
                                                     cp[:, p0: p0 + fw])
                            if not small:
                                fc = fc2td[:, (p0 % (2 * CH)):
                                           (p0 % (2 * CH)) + w]
                            for side in range(2):
                                coff = 2 * p0 + side * w
                                mf = small and side == 0
                                iou = PI.tile([H, 3 * CH], F32, tag="iou")
                                for k in range(3):
                                    nc.tensor.matmul(
                                        iou[:, k * G: k * G + w],
                                        wtdx[:, k * H:(k + 1) * H],
                                        xs[:, xoff + coff:
                                           xoff + coff + w],
                                        start=(k * G) % 512 == 0,
                                        stop=False,
                                        skip_group_check=mf)
                                for k in range(3):
                                    nc.tensor.matmul(
                                        iou[:, k * G: k * G + w],
                                        wtdh[:, k * H:(k + 1) * H],
                                        hs[:, xoff + coff:
                                           xoff + coff + w],
                                        start=False, stop=False,
                                        skip_group_check=mf)
                                for k in range(3):
                                    nc.tensor.matmul(
                                        iou[:, k * G: k * G + w],
                                        utd[:, k * H:(k + 1) * H],
                                        hp[:, p0: p0 + w],
                                        start=False, stop=not mf,
                                        skip_group_check=mf)
                                if mf:
                                    # parent forget gate rides in side A's
                                    # iou tile at [3w:4w]; bias via a K=1
                                    # ones matmul
                                    nc.tensor.matmul(
                                        iou[:, 3 * w:4 * w], uftd,
                                        hp[:, p0: p0 + w],
                                        start=(3 * w) % 512 == 0,
                                        stop=False, skip_group_check=True)
                                    nc.tensor.matmul(
                                        iou[:, 3 * w:4 * w], btd1,
                                        ones1[:, 0:w], start=False,
                                        stop=True, skip_group_check=True)
                                send = 4 * w if mf else 2 * G + w
                                sio = SIO.tile([H, 3 * CH], BF16,
                                               tag="sio")
                                nc.scalar.activation(
                                    out=sio[:, :send],
                                    in_=iou[:, 0:send],
                                    func=ActF.Sigmoid)
                                if mf:
                                    fc2td = FCL.tile([H, 2 * CH], BF16,
                                                     tag="fcl")
                                    nc.vector.tensor_mul(
                                        fc2td[:, :w], sio[:, 3 * w:4 * w],
                                        cp[:, p0: p0 + w])
                                    fc = fc2td[:, 0:w]
                                tud = W.tile([H, CH], BF16, tag="tud")
                                nc.vector.tensor_scalar(
                                    out=tud[:, :w],
                                    in0=sio[:, 2 * G:2 * G + w],
                                    scalar1=2.0, scalar2=-1.0,
                                    op0=Alu.mult, op1=Alu.add)
                                t1 = W.tile([H, CH], BF16, tag="t1")
                                nc.vector.tensor_mul(t1[:, :w], sio[:, :w],
                                                     tud[:, :w])
                                nc.vector.tensor_add(
                                    cw[:, coff: coff + w],
                                    t1[:, :w], fc)
                                pending.append((t, coff, w, G, sio))
                            yield
                        if n <= 2 * CH:
                            td_flush(pending)
                    if pending:
                        td_flush(pending)

                def gen():
                    yield from bu_gen()
                    yield from td_gen()

                return gen()

            gA = mk_stream(0)
            gB = mk_stream(1)
            # level-lockstep schedule: both streams walk the levels together.
            # Big levels pipeline chunk-interleaved; the small levels give two
            # INDEPENDENT serial chains whose semaphore latencies hide each
            # other.  L8 runs A-block then B-block so the DMA order (A's slab
            # first) keeps feeding compute.
            # Per-stream quanta: BU L8:8 L7:4 L6:2 L5..L0:1 each;
            # TD t0..t6: 1 each, t7: 2, t8: 4.
            counts = [4, 2, 1, 1, 1, 1, 1, 1,          # BU L7..L0
                      1, 1, 1, 1, 1, 1, 1, 2, 4]      # TD t0..t8
            sched = "A" * 8 + "B" * 8 + "".join("AB" * c for c in counts)
            for who in sched:
                g = gA if who == "A" else gB
                next(g, None)
            for g in (gA, gB):
                for _ in g:
                    pass

            nc.sync.dma_start(out=OUT[:], in_=outsb[:])

    _split_multi_waits(nc)
    return nc


def _perm_stream():
    """Per-stream node permutation: level-grouped chunk-local-split order.
    Entry = row index into the stream's [8176, 128] X slab."""
    trees = np.arange(TPCS, dtype=np.int64)
    heap = [np.zeros(TPCS, dtype=np.int64)]
    tree = [trees.copy()]
    for t in range(1, DEPTH + 1):
        ph, pt = heap[t - 1], tree[t - 1]
        nh, ntr = [], []
        for j in range(0, len(ph), CH):
            bh = ph[j:j + CH]
            bt = pt[j:j + CH]
            nh.append(2 * bh + 1)
            nh.append(2 * bh + 2)
            ntr.append(bt)
            ntr.append(bt)
        heap.append(np.concatenate(nh))
        tree.append(np.concatenate(ntr))
    parts = [tree[t] * M + heap[t] for t in range(DEPTH, -1, -1)]
    return np.concatenate(parts)


def _perm():
    ps = _perm_stream()
    return np.concatenate([ps, ps + NODES_S])


def kernel(**inputs):
    from concourse.bass_utils import run_bass_kernel_spmd

    X = np.asarray(inputs["X"], dtype=np.float32)
    W_iou_bu = np.asarray(inputs["W_iou_bu"], dtype=np.float32)
    U_iou_bu = np.asarray(inputs["U_iou_bu"], dtype=np.float32)
    Uf_bu = np.asarray(inputs["Uf_bu"], dtype=np.float32)
    bf_bu = np.asarray(inputs["bf_bu"], dtype=np.float32)
    W_iou_td = np.asarray(inputs["W_iou_td"], dtype=np.float32)
    U_iou_td = np.asarray(inputs["U_iou_td"], dtype=np.float32)
    Uf_td = np.asarray(inputs["Uf_td"], dtype=np.float32)
    bf_td = np.asarray(inputs["bf_td"], dtype=np.float32)

    bf16 = ml_dtypes.bfloat16
    # u-gate row blocks scaled by 2: the kernel computes
    # tanh(u) = 2*sigmoid(2u) - 1 on the vector engine
    s2 = np.ones((1, 3 * H), np.float32)
    s2[:, 2 * H:] = 2.0
    wpack = np.concatenate([
        W_iou_bu.T * s2, U_iou_bu.T * s2, Uf_bu.T,
        W_iou_td[:, :H].T * s2, W_iou_td[:, H:].T * s2, U_iou_td.T * s2,
        Uf_td.T,
    ], axis=1)
    bpack = np.stack([bf_bu, bf_td], axis=1)
    wpack_bf = wpack.astype(bf16)
    # row-0 extras for K=1 bias matmuls: [bf_bu | bf_td | ones]
    extras = np.zeros((H, 768), np.float32)
    extras[0, 0:128] = bf_bu
    extras[0, 128:256] = bf_td
    extras[0, 256:768] = 1.0
    extras_bf = extras.astype(bf16)
    shared = {
        "BPACK": np.ascontiguousarray(bpack, dtype=np.float32),
    }
    perm = _perm()
    in_maps = []
    for c in range(NCORES):
        slab = X[c * NC_NODES:(c + 1) * NC_NODES]
        xtc = slab[perm].T.astype(bf16)
        m = dict(shared)
        m["XTW"] = np.ascontiguousarray(
            np.concatenate([wpack_bf, extras_bf, xtc], axis=1))
        in_maps.append(m)

    if "nc" not in _NC_CACHE:
        _NC_CACHE["nc"] = _build_nc()
    nc = _NC_CACHE["nc"]

    trace = bool(os.environ.get("BIDI_TRACE"))
    if trace:
        sys.path.insert(0, "/root/problem/work")
        try:
            import ntff_hook
            ntff_hook.install()
        except Exception:
            trace = False
    res = run_bass_kernel_spmd(nc, in_maps, core_ids=list(range(NCORES)),
                               trace=trace)
    global LAST_EXEC_NS, LAST_TRACE
    LAST_EXEC_NS = res.exec_time_ns
    LAST_TRACE = res.instructions_and_trace

    out = np.empty((B, 2 * H), dtype=np.float32)
    for c in range(NCORES):
        o = res.results[c]["OUT"]          # [128, 64]
        out[c * TPC:(c + 1) * TPC, :H] = o[:, 0:32].T
        out[c * TPC:(c + 1) * TPC, H:] = o[:, 32:64].T
    return out


LAST_EXEC_NS = None
LAST_TRACE = None


# revision 21
# speedup vs baseline: 1.0265x; 1.0265x over previous
"""BiDiTreeLSTM forest kernel for 8 Trainium2 NeuronCores.

Strategy (data-parallel over trees, per the sharding hint):
  - 256 complete binary trees (depth 8, 511 nodes); 32 trees per core.
  - Everything on-device is FEATURE-MAJOR: [128 features on partitions,
    nodes on the free axis].  H == X == 128 exactly fills the partitions.
  - Host pre-permutes each core's nodes into a level-grouped "chunk-local
    split" order: level blocks stored t=8..0; within a level, the children
    of the parents in 512-column chunk j form two adjacent 512-column
    chunks 2j (left) and 2j+1 (right).  Every child/parent gather in both
    propagation passes is then a contiguous column slice and dependencies
    between levels are chunk-local.
  - Bottom-up then top-down level-synchronous ChildSum TreeLSTM per core,
    matmuls in bf16 (fp32 PSUM accumulate), elementwise in bf16 on the
    vector engine (2x/4x mode), gates on the scalar engine.
  - The scalar (ACT) engine is the bottleneck (~300ns fixed cost per
    instruction + ~0.85ns/col), so ACT instructions are merged:
      * tanh(u) = 2*sigmoid(2u) - 1 with the u-row blocks of W/U pre-scaled
        by 2 on the host, so i,o,u share ONE sigmoid over the contiguous
        PSUM iou tile; a 1-instruction vector affine (2x-1) recovers tanh.
      * tanh(c)/h of two adjacent chunks are deferred and flushed as ONE
        double-width tanh ("phase2 pairing").
      * top-down forget gates of two adjacent parent chunks share one
        sigmoid and one f*c multiply.
  - The 32 trees are split into TWO 16-tree streams emitted phase-shifted:
    the small tree levels (<=512 cols) are serial latency chains that
    leave every engine mostly idle, so one stream's big levels are
    interleaved into the other stream's serial middle.  Streams are
    independent; they share the PSUM pools (WAR deps serialize banks).
  - h0/c0 are zeros and b_iou_* are zeros by problem spec; the kernel
    exploits that (they do not affect the output otherwise).
Output per core: [128, 64] fp32, feature-major:
  [root h_bu (A|B 16+16) | leaf-mean h_td (A|B 16+16)];
host reassembles the [256, 256] result.
"""

import os
import sys

sys.path.insert(0, "/opt/trn_rl_repo")

import ml_dtypes
import numpy as np

import concourse.bass as bass
import concourse.mybir as mybir
import concourse.tile as tile

B = 256
DEPTH = 8
M = 511
H = 128
NCORES = 8
TPC = B // NCORES            # trees per core = 32
NS = 2                       # phase-shifted streams per core
TPCS = TPC // NS             # trees per stream = 16
NODES_S = TPCS * M           # 8176
NC_NODES = TPC * M           # 16352
CH = 512                     # moving-dim chunk (one fp32 PSUM bank)

NTS = {t: TPCS * (1 << t) for t in range(DEPTH + 1)}    # cols per level
OFFS = {}
_o = 0
for _t in range(DEPTH, -1, -1):                           # level 8 first
    OFFS[_t] = _o
    _o += NTS[_t]
assert _o == NODES_S

F32 = mybir.dt.float32
BF16 = mybir.dt.bfloat16
ActF = mybir.ActivationFunctionType
Alu = mybir.AluOpType

_NC_CACHE = {}


def _split_multi_waits(nc):
    """This container's walrus accepts at most ONE sync wait per
    instruction; Tile attaches several.  Insert single-wait NoOps."""
    n = 0
    for fn in nc.m.functions:
        for bb in fn.blocks:
            insts = bb.instructions
            new = []
            for inst in insts:
                si = inst.sync_info
                if si is not None and si.on_wait and len(si.on_wait) > 1:
                    waits = list(si.on_wait)
                    for j, w in enumerate(waits[:-1]):
                        new.append(mybir.InstNoOp(
                            name=f"{inst.name}_w{j}",
                            sync_info=mybir.SyncInfo(on_wait=[w], on_update=[]),
                            bass_nofuse=True,
                            engine=inst.engine,
                        ))
                        n += 1
                    si.on_wait = [waits[-1]]
                new.append(inst)
            if len(new) != len(insts):
                bb.instructions[:] = new
    return n


def _build_nc():
    nc = bass.Bass("TRN2")

    # weights prepended to X: [WBU | UBU | UFBU | WTDX | WTDH | UTD | UFTD |
    # row0-extras (bf_bu | bf_td | ones, for K=1 bias matmuls) | X nodes]
    # (u-gate row blocks pre-scaled by 2: tanh(u) = 2*sig(2u)-1)
    XW = 2176 + 768
    XTW = nc.dram_tensor("XTW", [H, XW + NC_NODES], BF16,
                         kind="ExternalInput")
    BPACK = nc.dram_tensor("BPACK", [H, 2], F32, kind="ExternalInput")
    OUT = nc.dram_tensor("OUT", [H, 64], F32, kind="ExternalOutput")

    with tile.TileContext(nc) as tc:
        with tc.tile_pool(name="persist", bufs=1) as P, \
             tc.tile_pool(name="ws", bufs=3) as W, \
             tc.tile_pool(name="sio", bufs=6) as SIO, \
             tc.tile_pool(name="fcl", bufs=4) as FCL, \
             tc.tile_pool(name="psiou", bufs=2, space="PSUM") as PI, \
             tc.tile_pool(name="psf", bufs=1, space="PSUM") as PF:

            # ---- persistent SBUF ----
            xtw = P.tile([H, XW + NC_NODES], BF16)
            wpk = xtw[:, 0:2176]
            xt = xtw[:, XW:]
            bpk = P.tile([H, 2], F32)
            # sync (HWDGE) queue: the level-8 weights + first chunk land
            # first so compute starts immediately; the rest of x streams on
            # the SWDGE (gpsimd) queue
            nc.sync.dma_start(out=xtw[:, 0:384], in_=XTW[:, 0:384])
            nc.sync.dma_start(out=xtw[:, XW:XW + 512],
                              in_=XTW[:, XW:XW + 512])
            nc.sync.dma_start(out=xtw[:, 384:XW], in_=XTW[:, 384:XW])
            nc.gpsimd.dma_start(out=bpk[:], in_=BPACK[:])
            for a, b in ((512, 2048), (2048, 4608), (4608, 8176),
                         (8176, 10240), (10240, 13312), (13312, NC_NODES)):
                nc.gpsimd.dma_start(out=xtw[:, XW + a:XW + b],
                                    in_=XTW[:, XW + a:XW + b])
            # K=1 matmul operands (partition row 0 only)
            bbu1 = xtw[0:1, 2176:2304]
            btd1 = xtw[0:1, 2304:2432]
            ones1 = xtw[0:1, 2432:2944]
            wbu = wpk[:, 0:384]
            ubu = wpk[:, 384:768]
            ufbu = wpk[:, 768:896]
            wtdx = wpk[:, 896:1280]
            wtdh = wpk[:, 1280:1664]
            utd = wpk[:, 1664:2048]
            uftd = wpk[:, 2048:2176]
            bfbu = bpk[:, 0:1]
            bftd = bpk[:, 1:2]

            hbu = P.tile([H, NC_NODES], BF16)     # all bottom-up h
            ca = P.tile([H, 8192], BF16)          # c ping (even levels)
            cb = P.tile([H, 4096], BF16)          # c pong (odd levels)
            ha = P.tile([H, 2048], BF16)          # td h ping (even, t<8)
            hb = P.tile([H, 4096], BF16)          # td h pong (odd)
            outsb = P.tile([H, 64], F32)

            def mk_stream(s):
                """Per-stream views and emission generators."""
                xs = xt[:, s * NODES_S:(s + 1) * NODES_S]
                hs = hbu[:, s * NODES_S:(s + 1) * NODES_S]
                ca_s = ca[:, s * 4096:(s + 1) * 4096]
                cb_s = cb[:, s * 2048:(s + 1) * 2048]
                ha_s = ha[:, s * 1024:(s + 1) * 1024]
                hb_s = hb[:, s * 2048:(s + 1) * 2048]

                def cbuf(t):
                    return ca_s if t % 2 == 0 else cb_s

                def hbuf(t):
                    return ha_s if t % 2 == 0 else hb_s

                def bu_flush(pend):
                    if len(pend) == 2 and pend[0][0] == pend[1][0] \
                            and pend[1][1] == pend[0][1] + pend[0][2]:
                        (t, c0, w, G, sa), (_, _, _, _, sb) = pend
                        tch = W.tile([H, 2 * CH], BF16, tag="tc")
                        nc.scalar.activation(out=tch[:, :2 * w],
                                             in_=cbuf(t)[:, c0: c0 + 2 * w],
                                             func=ActF.Tanh)
                        o = OFFS[t] + c0
                        nc.vector.tensor_mul(hs[:, o: o + w],
                                             sa[:, G:G + w], tch[:, :w])
                        nc.vector.tensor_mul(hs[:, o + w: o + 2 * w],
                                             sb[:, G:G + w], tch[:, w:2 * w])
                    else:
                        for t, c0, w, G, sa in pend:
                            tch = W.tile([H, 2 * CH], BF16, tag="tc")
                            nc.scalar.activation(out=tch[:, :w],
                                                 in_=cbuf(t)[:, c0: c0 + w],
                                                 func=ActF.Tanh)
                            nc.vector.tensor_mul(
                                hs[:, OFFS[t] + c0: OFFS[t] + c0 + w],
                                sa[:, G:G + w], tch[:, :w])
                            if t == 0:
                                nc.vector.tensor_mul(
                                    outsb[:, s * TPCS:(s + 1) * TPCS],
                                    sa[:, G:G + w], tch[:, :w])
                    pend.clear()

                def bu_gen():
                    pending = []
                    for t in range(DEPTH, -1, -1):
                        n = NTS[t]
                        xoff = OFFS[t]
                        cw = cbuf(t)
                        for c0 in range(0, n, CH):
                            if len(pending) == 2:
                                bu_flush(pending)
                            w = min(CH, n - c0)
                            G = w if w < CH else CH
                            iou = PI.tile([H, 3 * CH], F32, tag="iou")
                            for k in range(3):
                                nc.tensor.matmul(
                                    iou[:, k * G: k * G + w],
                                    wbu[:, k * H:(k + 1) * H],
                                    xs[:, xoff + c0: xoff + c0 + w],
                                    start=(k * G) % 512 == 0,
                                    stop=(t == DEPTH))
                            small = False  # f-merge regressed: adds K=1
                            # bias matmuls to the serial sigmoid dep chain
                            if t < DEPTH:
                                choff = OFFS[t + 1]
                                hl = hs[:, choff + 2 * c0:
                                        choff + 2 * c0 + w]
                                hr = hs[:, choff + 2 * c0 + w:
                                        choff + 2 * c0 + 2 * w]
                                cc = cbuf(t + 1)
                                if small:
                                    # forget-gate pre-activations ride in
                                    # the iou tile at [3w:5w] (bias added
                                    # via K=1 ones matmuls) so ONE sigmoid
                                    # covers i,o,u,fl,fr
                                    nc.tensor.matmul(
                                        iou[:, 3 * w:4 * w], ufbu, hl,
                                        start=(3 * w) % 512 == 0,
                                        stop=False, skip_group_check=True)
                                    nc.tensor.matmul(
                                        iou[:, 4 * w:5 * w], ufbu, hr,
                                        start=(4 * w) % 512 == 0,
                                        stop=False, skip_group_check=True)
                                    nc.tensor.matmul(
                                        iou[:, 3 * w:4 * w], bbu1,
                                        ones1[:, 0:w], start=False,
                                        stop=False, skip_group_check=True)
                                    nc.tensor.matmul(
                                        iou[:, 4 * w:5 * w], bbu1,
                                        ones1[:, 0:w], start=False,
                                        stop=True, skip_group_check=True)
                                else:
                                    psf = PF.tile([H, 2 * CH], F32,
                                                  tag="pf")
                                    nc.tensor.matmul(psf[:, 0:w], ufbu, hl,
                                                     start=True, stop=True)
                                    nc.tensor.matmul(psf[:, G:G + w],
                                                     ufbu, hr,
                                                     start=G % 512 == 0,
                                                     stop=True)
                                if w == n:
                                    # single-chunk level: U@(hl+hr) as two
                                    # matmuls (skips the htild hop)
                                    for k in range(3):
                                        nc.tensor.matmul(
                                            iou[:, k * G: k * G + w],
                                            ubu[:, k * H:(k + 1) * H],
                                            hl, start=False, stop=False,
                                            skip_group_check=small)
                                        nc.tensor.matmul(
                                            iou[:, k * G: k * G + w],
                                            ubu[:, k * H:(k + 1) * H],
                                            hr, start=False,
                                            stop=not small,
                                            skip_group_check=small)
                                else:
                                    htild = W.tile([H, CH], BF16,
                                                   tag="htild")
                                    nc.vector.tensor_add(htild[:, :w],
                                                         hl, hr)
                                    for k in range(3):
                                        nc.tensor.matmul(
                                            iou[:, k * G: k * G + w],
                                            ubu[:, k * H:(k + 1) * H],
                                            htild[:, :w],
                                            start=False, stop=True)
                            if t < DEPTH and not small:
                                # f sigmoid first: its psf inputs are ready
                                # before the U matmuls feeding the big
                                # sigmoid (in-order ACT queue)
                                f = W.tile([H, 2 * CH], BF16, tag="f")
                                nc.scalar.activation(out=f[:, :G + w],
                                                     in_=psf[:, :G + w],
                                                     func=ActF.Sigmoid,
                                                     bias=bfbu)
                            # ONE sigmoid over [i | o | 2u (| fl | fr)]
                            send = 5 * w if small else 2 * G + w
                            sio = SIO.tile([H, 3 * CH], BF16, tag="sio")
                            nc.scalar.activation(out=sio[:, :send],
                                                 in_=iou[:, 0:send],
                                                 func=ActF.Sigmoid)
                            if t < DEPTH:
                                fsl = sio[:, 3 * w:5 * w] if small \
                                    else f[:, :2 * w]
                                fc2 = W.tile([H, 2 * CH], BF16, tag="fc2")
                                nc.vector.tensor_mul(
                                    fc2[:, :2 * w], fsl,
                                    cc[:, 2 * c0: 2 * c0 + 2 * w])
                                cred = W.tile([H, CH], BF16, tag="cred")
                                nc.vector.tensor_add(cred[:, :w],
                                                     fc2[:, :w],
                                                     fc2[:, w:2 * w])
                            # tanh(u) = 2*sig(2u) - 1
                            tud = W.tile([H, CH], BF16, tag="tud")
                            nc.vector.tensor_scalar(
                                out=tud[:, :w],
                                in0=sio[:, 2 * G:2 * G + w],
                                scalar1=2.0, scalar2=-1.0,
                                op0=Alu.mult, op1=Alu.add)
                            cdst = cw[:, c0: c0 + w]
                            if t < DEPTH:
                                t1 = W.tile([H, CH], BF16, tag="t1")
                                nc.vector.tensor_mul(t1[:, :w], sio[:, :w],
                                                     tud[:, :w])
                                nc.vector.tensor_add(cdst, t1[:, :w],
                                                     cred[:, :w])
                            else:
                                nc.vector.tensor_mul(cdst, sio[:, :w],
                                                     tud[:, :w])
                            pending.append((t, c0, w, G, sio))
                            yield
                        if n <= 2 * CH:
                            bu_flush(pending)
                    if pending:
                        bu_flush(pending)

                def td_flush(pend):
                    paired = (len(pend) == 2 and pend[0][0] == pend[1][0]
                              and pend[1][1] == pend[0][1] + pend[0][2])
                    if paired:
                        (t, coff, w, G, sa), (_, _, _, _, sb) = pend
                        tch = W.tile([H, 2 * CH], BF16, tag="tc")
                        nc.scalar.activation(out=tch[:, :2 * w],
                                             in_=cbuf(t)[:, coff:
                                                         coff + 2 * w],
                                             func=ActF.Tanh)
                        if t < DEPTH:
                            hw_ = hbuf(t)
                            nc.vector.tensor_mul(hw_[:, coff: coff + w],
                                                 sa[:, G:G + w], tch[:, :w])
                            nc.vector.tensor_mul(
                                hw_[:, coff + w: coff + 2 * w],
                                sb[:, G:G + w], tch[:, w:2 * w])
                        else:
                            hn = W.tile([H, 2 * CH], BF16, tag="hn")
                            nc.vector.tensor_mul(hn[:, :w],
                                                 sa[:, G:G + w], tch[:, :w])
                            nc.vector.tensor_mul(hn[:, w:2 * w],
                                                 sb[:, G:G + w],
                                                 tch[:, w:2 * w])
                            # leaf-mean partial folded straight into the
                            # output accumulator
                            pi = coff // (2 * CH)
                            ob = outsb[:, 32 + s * TPCS:32 + (s + 1) * TPCS]
                            lred = W.tile([H, TPCS], F32, tag="lred")
                            nc.vector.reduce_sum(
                                out=lred[:],
                                in_=hn[:, :2 * w].rearrange(
                                    "p (k t) -> p t k", t=TPCS),
                                axis=mybir.AxisListType.X)
                            if pi == 0:
                                nc.vector.tensor_scalar_mul(
                                    ob, lred[:], 1.0 / 256.0)
                            else:
                                nc.vector.scalar_tensor_tensor(
                                    out=ob, in0=lred[:],
                                    scalar=1.0 / 256.0, in1=ob,
                                    op0=Alu.mult, op1=Alu.add)
                    else:
                        for t, coff, w, G, sa in pend:
                            tch = W.tile([H, 2 * CH], BF16, tag="tc")
                            nc.scalar.activation(out=tch[:, :w],
                                                 in_=cbuf(t)[:, coff:
                                                             coff + w],
                                                 func=ActF.Tanh)
                            nc.vector.tensor_mul(
                                hbuf(t)[:, coff: coff + w],
                                sa[:, G:G + w], tch[:, :w])
                    pend.clear()

                def td_gen():
                    pending = []
                    for t in range(0, DEPTH + 1):
                        n = NTS[t]
                        xoff = OFFS[t]
                        cw = cbuf(t)
                        if t == 0:
                            w = n  # 16
                            G = w
                            iou = PI.tile([H, 3 * CH], F32, tag="iou")
                            for k in range(3):
                                nc.tensor.matmul(
                                    iou[:, k * G: k * G + w],
                                    wtdx[:, k * H:(k + 1) * H],
                                    xs[:, xoff: xoff + w],
                                    start=(k == 0), stop=False)
                                nc.tensor.matmul(
                                    iou[:, k * G: k * G + w],
                                    wtdh[:, k * H:(k + 1) * H],
                                    hs[:, xoff: xoff + w],
                                    start=False, stop=True)
                            sio = SIO.tile([H, 3 * CH], BF16, tag="sio")
                            nc.scalar.activation(out=sio[:, :3 * w],
                                                 in_=iou[:, 0:3 * w],
                                                 func=ActF.Sigmoid)
                            tud = W.tile([H, CH], BF16, tag="tud")
                            nc.vector.tensor_scalar(
                                out=tud[:, :w],
                                in0=sio[:, 2 * G:2 * G + w],
                                scalar1=2.0, scalar2=-1.0,
                                op0=Alu.mult, op1=Alu.add)
                            nc.vector.tensor_mul(cw[:, 0:w], sio[:, :w],
                                                 tud[:, :w])
                            pending.append((0, 0, w, G, sio))
                            td_flush(pending)
                            yield
                            continue
                        half = n // 2
                        hp = hbuf(t - 1)
                        cp = cbuf(t - 1)
                        fc2td = None
                        for p0 in range(0, half, CH):
                            if len(pending) == 2:
                                td_flush(pending)
                            w = min(CH, half - p0)
                            G = w if w < CH else CH
                            small = False
                            if not small and p0 % (2 * CH) == 0:
                                # forget gates for parent chunks p0 and
                                # p0+CH share one sigmoid and one f*c mul
                                fw = min(2 * CH, half - p0)
                                psf = PF.tile([H, 2 * CH], F32, tag="pf")
                                nc.tensor.matmul(psf[:, 0:w], uftd,
                                                 hp[:, p0: p0 + w],
                                                 start=True, stop=True)
                                if fw > CH:
                                    nc.tensor.matmul(
                                        psf[:, CH:fw], uftd,
                                        hp[:, p0 + CH: p0 + fw],
                                        start=True, stop=True)
                                f = W.tile([H, 2 * CH], BF16, tag="f")
                                nc.scalar.activation(out=f[:, :fw],
                                                     in_=psf[:, :fw],
                                                     func=ActF.Sigmoid,
                                                     bias=bftd)
                                fc2td = FCL.tile([H, 2 * CH], BF16,
                                                 tag="fcl")
                                nc.vector.tensor_mul(fc2td[:, :fw],
                                                     f[:, :fw],
                                                     cp[:, p0: p0 + fw])
                            if not small:
                                fc = fc2td[:, (p0 % (2 * CH)):
                                           (p0 % (2 * CH)) + w]
                            for side in range(2):
                                coff = 2 * p0 + side * w
                                mf = small and side == 0
                                iou = PI.tile([H, 3 * CH], F32, tag="iou")
                                for k in range(3):
                                    nc.tensor.matmul(
                                        iou[:, k * G: k * G + w],
                                        wtdx[:, k * H:(k + 1) * H],
                                        xs[:, xoff + coff:
                                           xoff + coff + w],
                                        start=(k * G) % 512 == 0,
                                        stop=False,
                                        skip_group_check=mf)
                                for k in range(3):
                                    nc.tensor.matmul(
                                        iou[:, k * G: k * G + w],
                                        wtdh[:, k * H:(k + 1) * H],
                                        hs[:, xoff + coff:
                                           xoff + coff + w],
                                        start=False, stop=False,
                                        skip_group_check=mf)
                                for k in range(3):
                                    nc.tensor.matmul(
                                        iou[:, k * G: k * G + w],
                                        utd[:, k * H:(k + 1) * H],
                                        hp[:, p0: p0 + w],
                                        start=False, stop=not mf,
                                        skip_group_check=mf)
                                if mf:
                                    # parent forget gate rides in side A's
                                    # iou tile at [3w:4w]; bias via a K=1
                                    # ones matmul
                                    nc.tensor.matmul(
                                        iou[:, 3 * w:4 * w], uftd,
                                        hp[:, p0: p0 + w],
                                        start=(3 * w) % 512 == 0,
                                        stop=False, skip_group_check=True)
                                    nc.tensor.matmul(
                                        iou[:, 3 * w:4 * w], btd1,
                                        ones1[:, 0:w], start=False,
                                        stop=True, skip_group_check=True)
                                send = 4 * w if mf else 2 * G + w
                                sio = SIO.tile([H, 3 * CH], BF16,
                                               tag="sio")
                                nc.scalar.activation(
                                    out=sio[:, :send],
                                    in_=iou[:, 0:send],
                                    func=ActF.Sigmoid)
                                if mf:
                                    fc2td = FCL.tile([H, 2 * CH], BF16,
                                                     tag="fcl")
                                    nc.vector.tensor_mul(
                                        fc2td[:, :w], sio[:, 3 * w:4 * w],
                                        cp[:, p0: p0 + w])
                                    fc = fc2td[:, 0:w]
                                tud = W.tile([H, CH], BF16, tag="tud")
                                nc.vector.tensor_scalar(
                                    out=tud[:, :w],
                                    in0=sio[:, 2 * G:2 * G + w],
                                    scalar1=2.0, scalar2=-1.0,
                                    op0=Alu.mult, op1=Alu.add)
                                t1 = W.tile([H, CH], BF16, tag="t1")
                                nc.vector.tensor_mul(t1[:, :w], sio[:, :w],
                                                     tud[:, :w])
                                nc.vector.tensor_add(
                                    cw[:, coff: coff + w],
                                    t1[:, :w], fc)
                                pending.append((t, coff, w, G, sio))
                            yield
                        if n <= 2 * CH:
                            td_flush(pending)
                    if pending:
                        td_flush(pending)

                def gen():
                    yield from bu_gen()
                    yield from td_gen()

                return gen()

            gA = mk_stream(0)
            gB = mk_stream(1)
            # level-lockstep schedule: both streams walk the levels together.
            # Big levels pipeline chunk-interleaved; the small levels give two
            # INDEPENDENT serial chains whose semaphore latencies hide each
            # other.  L8 runs A-block then B-block so the DMA order (A's slab
            # first) keeps feeding compute.
            # Per-stream quanta: BU L8:8 L7:4 L6:2 L5..L0:1 each;
            # TD t0..t6: 1 each, t7: 2, t8: 4.
            counts = [4, 2, 1, 1, 1, 1, 1, 1,          # BU L7..L0
                      1, 1, 1, 1, 1, 1, 1, 2, 4]      # TD t0..t8
            sched = "A" * 8 + "B" * 8 + "".join("AB" * c for c in counts)
            for who in sched:
                g = gA if who == "A" else gB
                next(g, None)
            for g in (gA, gB):
                for _ in g:
                    pass

            nc.sync.dma_start(out=OUT[:], in_=outsb[:])

    _split_multi_waits(nc)
    return nc


def _perm_stream():
    """Per-stream node permutation: level-grouped chunk-local-split order.
    Entry = row index into the stream's [8176, 128] X slab."""
    trees = np.arange(TPCS, dtype=np.int64)
    heap = [np.zeros(TPCS, dtype=np.int64)]
    tree = [trees.copy()]
    for t in range(1, DEPTH + 1):
        ph, pt = heap[t - 1], tree[t - 1]
        nh, ntr = [], []
        for j in range(0, len(ph), CH):
            bh = ph[j:j + CH]
            bt = pt[j:j + CH]
            nh.append(2 * bh + 1)
            nh.append(2 * bh + 2)
            ntr.append(bt)
            ntr.append(bt)
        heap.append(np.concatenate(nh))
        tree.append(np.concatenate(ntr))
    parts = [tree[t] * M + heap[t] for t in range(DEPTH, -1, -1)]
    return np.concatenate(parts)


def _perm():
    ps = _perm_stream()
    return np.concatenate([ps, ps + NODES_S])


def kernel(**inputs):
    from concourse.bass_utils import run_bass_kernel_spmd

    X = np.asarray(inputs["X"], dtype=np.float32)
    W_iou_bu = np.asarray(inputs["W_iou_bu"], dtype=np.float32)
    U_iou_bu = np.asarray(inputs["U_iou_bu"], dtype=np.float32)
    Uf_bu = np.asarray(inputs["Uf_bu"], dtype=np.float32)
    bf_bu = np.asarray(inputs["bf_bu"], dtype=np.float32)
    W_iou_td = np.asarray(inputs["W_iou_td"], dtype=np.float32)
    U_iou_td = np.asarray(inputs["U_iou_td"], dtype=np.float32)
    Uf_td = np.asarray(inputs["Uf_td"], dtype=np.float32)
    bf_td = np.asarray(inputs["bf_td"], dtype=np.float32)

    bf16 = ml_dtypes.bfloat16
    # u-gate row blocks scaled by 2: the kernel computes
    # tanh(u) = 2*sigmoid(2u) - 1 on the vector engine
    s2 = np.ones((1, 3 * H), np.float32)
    s2[:, 2 * H:] = 2.0
    wpack = np.concatenate([
        W_iou_bu.T * s2, U_iou_bu.T * s2, Uf_bu.T,
        W_iou_td[:, :H].T * s2, W_iou_td[:, H:].T * s2, U_iou_td.T * s2,
        Uf_td.T,
    ], axis=1)
    bpack = np.stack([bf_bu, bf_td], axis=1)
    wpack_bf = wpack.astype(bf16)
    # row-0 extras for K=1 bias matmuls: [bf_bu | bf_td | ones]
    extras = np.zeros((H, 768), np.float32)
    extras[0, 0:128] = bf_bu
    extras[0, 128:256] = bf_td
    extras[0, 256:768] = 1.0
    extras_bf = extras.astype(bf16)
    shared = {
        "BPACK": np.ascontiguousarray(bpack, dtype=np.float32),
    }
    perm = _perm()
    in_maps = []
    for c in range(NCORES):
        slab = X[c * NC_NODES:(c + 1) * NC_NODES]
        xtc = slab[perm].T.astype(bf16)
        m = dict(shared)
        m["XTW"] = np.ascontiguousarray(
            np.concatenate([wpack_bf, extras_bf, xtc], axis=1))
        in_maps.append(m)

    if "nc" not in _NC_CACHE:
        _NC_CACHE["nc"] = _build_nc()
    nc = _NC_CACHE["nc"]

    trace = bool(os.environ.get("BIDI_TRACE"))
    if trace:
        sys.path.insert(0, "/root/problem/work")
        try:
            import ntff_hook
            ntff_hook.install()
        except Exception:
            trace = False
    res = run_bass_kernel_spmd(nc, in_maps, core_ids=list(range(NCORES)),
                               trace=trace)
    global LAST_EXEC_NS, LAST_TRACE
    LAST_EXEC_NS = res.exec_time_ns
    LAST_TRACE = res.instructions_and_trace

    out = np.empty((B, 2 * H), dtype=np.float32)
    for c in range(NCORES):
        o = res.results[c]["OUT"]          # [128, 64]
        out[c * TPC:(c + 1) * TPC, :H] = o[:, 0:32].T
        out[c * TPC:(c + 1) * TPC, H:] = o[:, 32:64].T
    return out


LAST_EXEC_NS = None
LAST_TRACE = None
